# revision 11
# baseline (speedup 1.0000x reference)
"""Trainium2 Bass kernel for nn_Conv_M_49409303773352.

Strategy (data-parallel over batch x H-halves -> 8 shards):
  Per position p=(b,h,w): feat = [x-patches(576), m-patches(576)] (3x3, edge pad)
  w1 = feat@W1+b1 [576]; w2 = feat@W2+b2 [4096]
  yr_c = sum_k yp*w1 ; mr_c = sum_k |mp*w1| ; sr_c = sum_k |sp*w1|   (mp,sp>=0)
  y_o  = sum_c yr_c*w2[c,o] ; m_y = (sum_c |mr_c*w2|)/(sum_c |sr_c*w2|)

Wire-traffic-lean layout (the axon H2D link is the bottleneck, ~40MB/s):
  - Only raw padded rows ship to the device (xm stacked [2C, ROWS+2, W+2] and
    s [C, ROWS+2, W+2], bf16). The per-position patch tensors ([pos, C*K2],
    9x redundant) are built ON DEVICE with tensor-engine transposes per tap
    (bf16 PSUM out, batched PSUM->SBUF copies on the scalar engine).
  - W1/W2 (tap-gathered, col-permuted) are column-sharded 8 ways on the wire
    and reassembled on device with an HBM-HBM AllGather before the main loop.
  - Outputs return as fp16.
Patch/W1-col layout is k-major (f = k*C + c) so each tap's transpose lands
contiguously; the k-reduction reads stride-C views. GEMM contraction over
channels (64 x + 64 m stacked = 128 partitions) per 3x3 tap k, accumulating in
PSUM; biases folded in via a K=1 ones-row matmul. All data bf16 on wire,
fp32 accumulation.

DMA discipline: DMA completion semaphores tick +16 per transfer and were
observed to corrupt data once a queue's running count crosses 2^13 (512
transfers) within a program (deterministic bad row at the crossing point
when patches were DMA-transposed: 21 DMAs/row). This version issues only 4
DMAs/row on one queue (~280 total), comfortably inside the envelope the
(correct) baseline ran in.
"""
import sys
sys.path.insert(0, '/opt/trn_rl_repo')
import os
import numpy as np
import ml_dtypes

# persistent XLA compilation cache: the PJRT wrapper around the NEFF is
# recompiled on every run_bass_via_pjrt call (fresh closure -> jit cache
# miss); the disk cache turns that ~1.1s recompile into a ~0.3s load
import jax
jax.config.update("jax_compilation_cache_dir", "/tmp/jax_comp_cache")
jax.config.update("jax_persistent_cache_min_entry_size_bytes", -1)
jax.config.update("jax_persistent_cache_min_compile_time_secs", 0.0)

import concourse.bass as bass
import concourse.mybir as mybir
import concourse.tile as tile
from concourse.tile import TileContext
from concourse.vector_clock import ScopedClock
from concourse.bass_utils import run_bass_kernel_spmd

BF = ml_dtypes.bfloat16
BF_DT = mybir.dt.bfloat16
F32 = mybir.dt.float32
F16 = mybir.dt.float16

B, C, H, W = 4, 64, 128, 128
ROWS = int(os.environ.get("KERNEL_ROWS", "64"))   # output rows per core
N_CORES = 8
K2 = 9
F1 = 576          # K2*C
F2 = 4096         # C*C
F1S = F1 // N_CORES   # 72  per-rank W1 column shard
F2S = F2 // N_CORES   # 512 per-rank W2 column shard


# ---- walrus only accepts ONE sem wait per instruction: split the final drain
def _split_drain_and_barrier(self, tick_clock, wait_clock):
    nc = self.nc
    probe = nc.sync.nop()
    wait_clock.add_sem_waits(probe.ins, ScopedClock({None: tick_clock.global_clock}))
    waits = list(probe.ins.sync_info.on_wait)
    if len(waits) > 1:
        probe.ins.sync_info.on_wait = waits[:1]
        for w in waits[1:]:
            extra = nc.sync.nop()
            extra.ins.sync_info = probe.ins.sync_info.__class__(
                on_wait=[w], on_update=[])
    nc.sync.drain()
    nc.all_engine_barrier()
    assert self.sems is not None
    popped = nc._tile_sem_poison_stack.pop()
    assert popped is self._sem_poison
    nc.clear_and_free_semaphores(list(self.sems.allocated().values()))
    nc.all_engine_barrier()


tile.TileContext._drain_and_barrier = _split_drain_and_barrier


def _split_multi_sync(nc):
    """Walrus accepts one sync wait (and update) per instruction: hoist extras
    onto same-engine nops inserted just before (waits) / after (updates)."""
    def make_nop(engine, si_cls, waits=(), updates=()):
        bi = nc.engines[engine].nop()
        blk = nc.cur_bb.bb
        assert blk.instructions[-1] is bi.ins
        blk.instructions.pop()
        bi.ins.sync_info = si_cls(on_wait=list(waits), on_update=list(updates))
        return bi.ins

    for blk in nc.m.functions[0].blocks:
        out = []
        for inst in blk.instructions:
            si = getattr(inst, "sync_info", None)
            if si is None:
                out.append(inst)
                continue
            waits = list(si.on_wait or [])
            updates = list(si.on_update or [])
            extra_w = waits[:-1] if len(waits) > 1 else []
            extra_u = updates[1:] if len(updates) > 1 else []
            if extra_w:
                for w in extra_w:
                    out.append(make_nop(inst.engine, si.__class__, waits=[w]))
                si.on_wait = waits[-1:]
            out.append(inst)
            if extra_u:
                assert inst.opcode not in (
                    "DMACopy", "DMATranspose", "DmaTransposeAnt"), \
                    "cannot defer DMA completion updates"
                si.on_update = updates[:1]
                for u in extra_u:
                    out.append(make_nop(inst.engine, si.__class__, updates=[u]))
        blk.instructions[:] = out


# flat element offsets of each logical input inside the packed "blob" input
# (one array per core: fewer transfers and a smaller PJRT wrapper)
_SEGS = (("xm", (2 * C, ROWS + 2, W + 2)),
         ("s", (C, ROWS + 2, W + 2)),
         ("w1s", (K2, 128, F1S)),
         ("w2s", (K2, 128, F2S)),
         ("b1", (1, F1)),
         ("b2", (1, F2)),
         ("ident", (128, 128)))
_OFFS = {}
_off = 0
for _nm, _shp in _SEGS:
    _OFFS[_nm] = _off
    _off += int(np.prod(_shp))
BLOB_N = _off


def build_program():
    nc = bass.Bass(num_devices=N_CORES)
    blob_d = nc.dram_tensor("blob", [BLOB_N], BF_DT, kind="ExternalInput")

    def seg(nm):
        shp = dict(_SEGS)[nm]
        n = int(np.prod(shp))
        flat = blob_d[_OFFS[nm]:_OFFS[nm] + n]
        names = " ".join(f"d{i}" for i in range(len(shp)))
        kw = {f"d{i}": int(s) for i, s in enumerate(shp[:-1])}
        return flat.rearrange(f"({names}) -> {names}", **kw)

    xm_d = seg("xm")
    s_d = seg("s")
    w1s_d = seg("w1s")
    w2s_d = seg("w2s")
    b1_d = seg("b1")
    b2_d = seg("b2")
    id_d = seg("ident")
    # single merged output: d2h has ~0.2s fixed cost per fetched array
    ym_d = nc.dram_tensor("ym", [ROWS, 2, W, C], F16, kind="ExternalOutput")

    # collectives can't read IO tensors: bounce shards to internal DRAM first
    w1tmp = nc.dram_tensor("w1tmp", [K2, 128, F1S], BF_DT)
    w2tmp = nc.dram_tensor("w2tmp", [K2, 128, F2S], BF_DT)
    w1g = nc.dram_tensor("w1g", [N_CORES, K2, 128, F1S], BF_DT)
    w2g = nc.dram_tensor("w2g", [N_CORES, K2, 128, F2S], BF_DT)

    groups = [[i for i in range(N_CORES)]]
    sem = nc.alloc_semaphore("ag_sem")
    nc.sync.dma_start(out=w1tmp[:, :, :], in_=w1s_d).then_inc(sem, 16)
    nc.sync.dma_start(out=w2tmp[:, :, :], in_=w2s_d).then_inc(sem, 16)
    nc.gpsimd.wait_ge(sem, 32)
    nc.gpsimd.collective_compute(
        "AllGather", mybir.AluOpType.bypass, replica_groups=groups,
        ins=[w1tmp[:, :, :].opt()], outs=[w1g[:, :, :, :].opt()]).then_inc(sem, 1)
    nc.gpsimd.collective_compute(
        "AllGather", mybir.AluOpType.bypass, replica_groups=groups,
        ins=[w2tmp[:, :, :].opt()], outs=[w2g[:, :, :, :].opt()]).then_inc(sem, 1)
    nc.gpsimd.wait_ge(sem, 34)
    nc.all_engine_barrier()
    nc.clear_and_free_semaphores([sem])
    nc.all_engine_barrier()

    # tap groups for batched transpose->copy; PSUM spends a 32-bit word
    # per element even for bf16, so 4x128 per bank
    TGROUPS = ((0, 4), (4, 4), (8, 1))

    with TileContext(nc) as tc:
        with (
            tc.tile_pool(name="wts", bufs=1) as wts,
            tc.tile_pool(name="rows", bufs=4) as rows,
            tc.tile_pool(name="pats", bufs=3) as pats,
            tc.tile_pool(name="mid", bufs=4) as mid,
            tc.tile_pool(name="sml", bufs=3) as sml,
            tc.tile_pool(name="psw1", bufs=1, space="PSUM") as psw1,
            tc.tile_pool(name="psw2", bufs=2, space="PSUM") as psw2,
            tc.tile_pool(name="pst", bufs=2, space="PSUM") as pst,
        ):
            # gathered weights: f1 = rank*F1S + j contiguous; w2 ranks as
            # an explicit axis (rank r holds output-cols [F2S*r, F2S*(r+1)))
            w1k = wts.tile([128, K2, F1], BF_DT)
            w1kv = w1k[:, :, :].rearrange("p k (r j) -> p k r j", r=N_CORES)
            w2k = wts.tile([128, K2, N_CORES, F2S], BF_DT)
            for r in range(N_CORES):
                nc.sync.dma_start(
                    out=w1kv[:, :, r, :],
                    in_=w1g[r, :, :, :].rearrange("k p j -> p k j"))
                nc.sync.dma_start(
                    out=w2k[:, :, r, :],
                    in_=w2g[r, :, :, :].rearrange("k p f -> p k f"))
            b1s = wts.tile([1, F1], BF_DT)
            nc.sync.dma_start(out=b1s, in_=b1_d)
            b2s = wts.tile([1, F2], BF_DT)
            nc.sync.dma_start(out=b2s, in_=b2_d)
            ident = wts.tile([128, 128], BF_DT)
            nc.sync.dma_start(out=ident, in_=id_d)
            ones = wts.tile([1, 128], BF_DT)
            nc.vector.memset(ones, 1.0)

            for h in range(ROWS):
                xmr = rows.tile([128, 3, W + 2], BF_DT)
                nc.sync.dma_start(out=xmr, in_=xm_d[:, h:h + 3, :])
                sr = rows.tile([64, 3, W + 2], BF_DT, tag="srow")
                nc.sync.dma_start(out=sr, in_=s_d[:, h:h + 3, :])

                # on-device patch build: per tap, PE-transpose the padded row
                # slice into [pos, chan] (bf16 PSUM), then batched copy to
                # SBUF; k-major free layout
                xm_pt = pats.tile([128, K2, 128], BF_DT)
                s_pt = pats.tile([128, K2, C], BF_DT)
                for g0, gn in TGROUPS:
                    ptx = pst.tile([128, 4, 128], BF_DT, tag="pt")
                    for j in range(gn):
                        kh, kw = divmod(g0 + j, 3)
                        nc.tensor.transpose(
                            ptx[:, j, :], xmr[:, kh, kw:kw + 128], ident)
                    nc.scalar.copy(out=xm_pt[:, g0:g0 + gn, :],
                                   in_=ptx[:, 0:gn, :])
                for g0, gn in TGROUPS:
                    pts = pst.tile([128, 4, 128], BF_DT, tag="pt")
                    for j in range(gn):
                        kh, kw = divmod(g0 + j, 3)
                        nc.tensor.transpose(
                            pts[:, j, 0:C], sr[:, kh, kw:kw + 128],
                            ident[0:C, 0:C])
                    nc.scalar.copy(out=s_pt[:, g0:g0 + gn, :],
                                   in_=pts[:, 0:gn, 0:C])

                # ---- w1 = feat @ W1 + b1  -> PSUM [128 pos, 576]
                ps1 = psw1.tile([128, F1], F32)
                for lo, hi in ((0, 512), (512, F1)):
                    for k in range(K2):
                        kh, kw = divmod(k, 3)
                        nc.tensor.matmul(
                            ps1[:, lo:hi], xmr[:, kh, kw:kw + 128],
                            w1k[:, k, lo:hi], start=(k == 0), stop=False)
                    nc.tensor.matmul(ps1[:, lo:hi], ones[0:1, :],
                                     b1s[0:1, lo:hi], start=False, stop=True)
                w1b = mid.tile([128, F1], BF_DT)
                nc.scalar.copy(out=w1b, in_=ps1)
                w1v = w1b[:, :].rearrange("p (k c) -> p k c", c=C)

                # ---- yr/mr/sr: per-position reduce over the 9 taps
                rmap = []
                for ci, (pat, absv) in enumerate(
                        ((xm_pt[:, :, 0:C], None),
                         (xm_pt[:, :, C:2 * C], True),
                         (s_pt[:, :, :], True))):
                    t1 = mid.tile([128, K2, C], BF_DT)
                    nc.gpsimd.tensor_mul(t1, pat, w1v)
                    red = sml.tile([128, C], F32, tag=f"red{ci}")
                    nc.vector.tensor_reduce(
                        out=red, in_=t1[:, :, :].rearrange("p k c -> p c k"),
                        axis=mybir.AxisListType.X, op=mybir.AluOpType.add,
                        apply_absolute_value=absv)
                    redb = sml.tile([128, C], BF_DT, tag=f"redb{ci}")
                    nc.scalar.copy(out=redb, in_=red)
                    rmap.append(redb)
                yrb, mrb, srb = rmap

                y_acc = sml.tile([128, C], F32)
                m_acc = sml.tile([128, C], F32)
                s_acc = sml.tile([128, C], F32)

                # ---- w2 = feat @ W2 + b2, 4 chunks of 1024 cols ([o,c] layout)
                for q in range(4):
                    ps2 = psw2.tile([128, 1024], F32)
                    for j2 in range(2):
                        r = q * 2 + j2
                        for k in range(K2):
                            kh, kw = divmod(k, 3)
                            nc.tensor.matmul(
                                ps2[:, j2 * 512:(j2 + 1) * 512],
                                xmr[:, kh, kw:kw + 128],
                                w2k[:, k, r, :], start=(k == 0), stop=False)
                        nc.tensor.matmul(
                            ps2[:, j2 * 512:(j2 + 1) * 512], ones[0:1, :],
                            b2s[0:1, r * F2S:(r + 1) * F2S],
                            start=False, stop=True)
                    w2b = mid.tile([128, 1024], BF_DT)
                    nc.scalar.copy(out=w2b, in_=ps2)
                    w2v = w2b[:, :].rearrange("p (o c) -> p o c", c=C)
                    for redb, acc, absv in ((yrb, y_acc, None),
                                            (mrb, m_acc, True),
                                            (srb, s_acc, True)):
                        t2 = mid.tile([128, 16, C], BF_DT)
                        bcast = redb[:, :].rearrange(
                            "p (o c) -> p o c", o=1).to_broadcast([128, 16, C])
                        # s-chain always on gpsimd; m-chain alternates to
                        # balance DVE vs gpsimd busy time
                        on_q7 = redb is srb or (redb is mrb and q % 2 == 0)
                        eng = nc.gpsimd if on_q7 else nc.vector
                        eng.tensor_mul(t2, w2v, bcast)
                        nc.vector.tensor_reduce(
                            out=acc[:, q * 16:(q + 1) * 16], in_=t2,
                            axis=mybir.AxisListType.X, op=mybir.AluOpType.add,
                            apply_absolute_value=absv)

                srec = sml.tile([128, C], F32)
                nc.vector.reciprocal(out=srec, in_=s_acc)
                my_t = sml.tile([128, C], F16)
                nc.vector.tensor_mul(my_t, m_acc, srec)
                y16 = sml.tile([128, C], F16)
                nc.scalar.copy(out=y16, in_=y_acc)
                nc.sync.dma_start(out=ym_d[h, 0, :, :], in_=y16)
                nc.sync.dma_start(out=ym_d[h, 1, :, :], in_=my_t)
    _split_multi_sync(nc)
    return nc


def _row_gather(Wm, k):
    # rows of W (1152) feeding tap k for channels [x 0..63, m 0..63]
    idx = np.concatenate([np.arange(64) * 9 + k, 576 + np.arange(64) * 9 + k])
    return Wm[idx]


def kernel(x, m, s, W1, b1, W2, b2):
    x = np.asarray(x, np.float32); m = np.asarray(m, np.float32)
    s = np.asarray(s, np.float32)
    W1 = np.asarray(W1, np.float32); W2 = np.asarray(W2, np.float32)
    b1 = np.asarray(b1, np.float32); b2 = np.asarray(b2, np.float32)

    # W1 cols permuted from [c,k] to [k,c] (k-major matches on-device patches)
    W1p = W1.reshape(1152, C, K2).transpose(0, 2, 1).reshape(1152, F1)
    b1p = b1.reshape(C, K2).T.reshape(1, F1).astype(BF)
    # W2 cols permuted from [c,o] to [o,c]; bias likewise
    W2p = W2.reshape(1152, C, C).transpose(0, 2, 1).reshape(1152, F2)
    b2p = b2.reshape(C, C).T.reshape(1, F2).astype(BF)
    w1k = np.stack([_row_gather(W1p.astype(BF), k) for k in range(K2)])
    w2k = np.stack([_row_gather(W2p.astype(BF), k) for k in range(K2)])

    xmp = np.pad(np.concatenate([x, m], axis=1),
                 ((0, 0), (0, 0), (1, 1), (1, 1)), mode='edge').astype(BF)
    sp = np.pad(s, ((0, 0), (0, 0), (1, 1), (1, 1)), mode='edge').astype(BF)
    ident = np.eye(128, dtype=BF)

    in_maps = []
    shards = []
    for core in range(N_CORES):
        b, half = divmod(core, 2)
        h0 = half * (H // 2)
        shards.append((b, h0))
        blob = np.concatenate([
            xmp[b, :, h0:h0 + ROWS + 2, :].ravel(),
            sp[b, :, h0:h0 + ROWS + 2, :].ravel(),
            w1k[:, :, core * F1S:(core + 1) * F1S].ravel(),
            w2k[:, :, core * F2S:(core + 1) * F2S].ravel(),
            b1p.ravel(), b2p.ravel(), ident.ravel(),
        ])
        assert blob.shape[0] == BLOB_N
        in_maps.append({"blob": blob})

    nc = build_program()
    res = run_bass_kernel_spmd(nc, in_maps, core_ids=list(range(N_CORES)),
                               trace=False)
    if os.environ.get("KERNEL_TIME"):
        # no NTFF profiling in this axon build: approximate device time by
        # wall-timing a repeat execution (includes host I/O, so upper bound)
        import time
        from concourse import bass2jax
        t0 = time.time()
        bass2jax.run_bass_via_pjrt(nc, in_maps, n_cores=N_CORES)
        with open("/tmp/kernel_exec_time.txt", "w") as f:
            f.write(str(int((time.time() - t0) * 1e9)))

    y = np.zeros((B, C, H, W), np.float32)
    m_y = np.zeros((B, C, H, W), np.float32)
    for core, (b, h0) in enumerate(shards):
        ym = res.results[core]["ym"]
        y[b, :, h0:h0 + ROWS, :] = ym[:, 0].transpose(2, 0, 1).astype(np.float32)
        m_y[b, :, h0:h0 + ROWS, :] = ym[:, 1].transpose(2, 0, 1).astype(np.float32)
    return y, m_y, np.ones_like(m_y)


# revision 12
# speedup vs baseline: 1.0554x; 1.0554x over previous
"""Trainium2 Bass kernel for nn_Conv_M_49409303773352.

Strategy (data-parallel over batch x H-halves -> 8 shards):
  Per position p=(b,h,w): feat = [x-patches(576), m-patches(576)] (3x3, edge pad)
  w1 = feat@W1+b1 [576]; w2 = feat@W2+b2 [4096]
  yr_c = sum_k yp*w1 ; mr_c = sum_k |mp*w1| ; sr_c = sum_k |sp*w1|   (mp,sp>=0)
  y_o  = sum_c yr_c*w2[c,o] ; m_y = (sum_c |mr_c*w2|)/(sum_c |sr_c*w2|)

Wire-traffic-lean layout (the axon H2D link is the bottleneck, ~40MB/s):
  - Only raw padded rows ship to the device (xm stacked [2C, ROWS+2, W+2] and
    s [C, ROWS+2, W+2], bf16). The per-position patch tensors ([pos, C*K2],
    9x redundant) are built ON DEVICE with tensor-engine transposes per tap
    (bf16 PSUM out, batched PSUM->SBUF copies on the scalar engine).
  - W1/W2 (tap-gathered, col-permuted) are column-sharded 8 ways on the wire
    and reassembled on device with an HBM-HBM AllGather before the main loop.
  - Outputs return as fp16.
Patch/W1-col layout is k-major (f = k*C + c) so each tap's transpose lands
contiguously; the k-reduction reads stride-C views. GEMM contraction over
channels (64 x + 64 m stacked = 128 partitions) per 3x3 tap k, accumulating in
PSUM; biases folded in via a K=1 ones-row matmul. All data bf16 on wire,
fp32 accumulation.

DMA discipline: DMA completion semaphores tick +16 per transfer and were
observed to corrupt data once a queue's running count crosses 2^13 (512
transfers) within a program (deterministic bad row at the crossing point
when patches were DMA-transposed: 21 DMAs/row). This version issues only 4
DMAs/row on one queue (~280 total), comfortably inside the envelope the
(correct) baseline ran in.
"""
import sys
sys.path.insert(0, '/opt/trn_rl_repo')
import os
import numpy as np
import ml_dtypes

# persistent XLA compilation cache: the PJRT wrapper around the NEFF is
# recompiled on every run_bass_via_pjrt call (fresh closure -> jit cache
# miss); the disk cache turns that ~1.1s recompile into a ~0.3s load
import jax
try:
    jax.config.update("jax_compilation_cache_dir", "/tmp/jax_comp_cache")
    jax.config.update("jax_persistent_cache_min_entry_size_bytes", -1)
    jax.config.update("jax_persistent_cache_min_compile_time_secs", 0.0)
except Exception:
    pass  # older jax without these flags: correctness unaffected

import concourse.bass as bass
import concourse.mybir as mybir
import concourse.tile as tile
from concourse.tile import TileContext
from concourse.vector_clock import ScopedClock
from concourse.bass_utils import run_bass_kernel_spmd

BF = ml_dtypes.bfloat16
BF_DT = mybir.dt.bfloat16
F32 = mybir.dt.float32
F16 = mybir.dt.float16

B, C, H, W = 4, 64, 128, 128
ROWS = int(os.environ.get("KERNEL_ROWS", "64"))   # output rows per core
N_CORES = 8
K2 = 9
F1 = 576          # K2*C
F2 = 4096         # C*C
F1S = F1 // N_CORES   # 72  per-rank W1 column shard
F2S = F2 // N_CORES   # 512 per-rank W2 column shard


# ---- walrus only accepts ONE sem wait per instruction: split the final drain
def _split_drain_and_barrier(self, tick_clock, wait_clock):
    nc = self.nc
    probe = nc.sync.nop()
    wait_clock.add_sem_waits(probe.ins, ScopedClock({None: tick_clock.global_clock}))
    waits = list(probe.ins.sync_info.on_wait)
    if len(waits) > 1:
        probe.ins.sync_info.on_wait = waits[:1]
        for w in waits[1:]:
            extra = nc.sync.nop()
            extra.ins.sync_info = probe.ins.sync_info.__class__(
                on_wait=[w], on_update=[])
    nc.sync.drain()
    nc.all_engine_barrier()
    assert self.sems is not None
    popped = nc._tile_sem_poison_stack.pop()
    assert popped is self._sem_poison
    nc.clear_and_free_semaphores(list(self.sems.allocated().values()))
    nc.all_engine_barrier()


tile.TileContext._drain_and_barrier = _split_drain_and_barrier


def _split_multi_sync(nc):
    """Walrus accepts one sync wait (and update) per instruction: hoist extras
    onto same-engine nops inserted just before (waits) / after (updates)."""
    def make_nop(engine, si_cls, waits=(), updates=()):
        bi = nc.engines[engine].nop()
        blk = nc.cur_bb.bb
        assert blk.instructions[-1] is bi.ins
        blk.instructions.pop()
        bi.ins.sync_info = si_cls(on_wait=list(waits), on_update=list(updates))
        return bi.ins

    for blk in nc.m.functions[0].blocks:
        out = []
        for inst in blk.instructions:
            si = getattr(inst, "sync_info", None)
            if si is None:
                out.append(inst)
                continue
            waits = list(si.on_wait or [])
            updates = list(si.on_update or [])
            extra_w = waits[:-1] if len(waits) > 1 else []
            extra_u = updates[1:] if len(updates) > 1 else []
            if extra_w:
                for w in extra_w:
                    out.append(make_nop(inst.engine, si.__class__, waits=[w]))
                si.on_wait = waits[-1:]
            out.append(inst)
            if extra_u:
                assert inst.opcode not in (
                    "DMACopy", "DMATranspose", "DmaTransposeAnt"), \
                    "cannot defer DMA completion updates"
                si.on_update = updates[:1]
                for u in extra_u:
                    out.append(make_nop(inst.engine, si.__class__, updates=[u]))
        blk.instructions[:] = out


# flat element offsets of each logical input inside the packed "blob" input
# (one array per core: fewer transfers and a smaller PJRT wrapper)
_SEGS = (("xm", (2 * C, ROWS + 2, W + 2)),
         ("s", (C, ROWS + 2, W + 2)),
         ("w1s", (K2, 128, F1S)),
         ("w2s", (K2, 128, F2S)),
         ("b1", (1, F1)),
         ("b2", (1, F2)),
         ("ident", (128, 128)))
_OFFS = {}
_off = 0
for _nm, _shp in _SEGS:
    _OFFS[_nm] = _off
    _off += int(np.prod(_shp))
BLOB_N = _off


def build_program():
    nc = bass.Bass(num_devices=N_CORES)
    blob_d = nc.dram_tensor("blob", [BLOB_N], BF_DT, kind="ExternalInput")

    def seg(nm):
        shp = dict(_SEGS)[nm]
        n = int(np.prod(shp))
        flat = blob_d[_OFFS[nm]:_OFFS[nm] + n]
        names = " ".join(f"d{i}" for i in range(len(shp)))
        kw = {f"d{i}": int(s) for i, s in enumerate(shp[:-1])}
        return flat.rearrange(f"({names}) -> {names}", **kw)

    xm_d = seg("xm")
    s_d = seg("s")
    w1s_d = seg("w1s")
    w2s_d = seg("w2s")
    b1_d = seg("b1")
    b2_d = seg("b2")
    id_d = seg("ident")
    # single merged output: d2h has ~0.2s fixed cost per fetched array
    ym_d = nc.dram_tensor("ym", [ROWS, 2, W, C], F16, kind="ExternalOutput")

    # collectives can't read IO tensors: bounce shards to internal DRAM first
    w1tmp = nc.dram_tensor("w1tmp", [K2, 128, F1S], BF_DT)
    w2tmp = nc.dram_tensor("w2tmp", [K2, 128, F2S], BF_DT)
    w1g = nc.dram_tensor("w1g", [N_CORES, K2, 128, F1S], BF_DT)
    w2g = nc.dram_tensor("w2g", [N_CORES, K2, 128, F2S], BF_DT)

    groups = [[i for i in range(N_CORES)]]
    sem = nc.alloc_semaphore("ag_sem")
    nc.sync.dma_start(out=w1tmp[:, :, :], in_=w1s_d).then_inc(sem, 16)
    nc.sync.dma_start(out=w2tmp[:, :, :], in_=w2s_d).then_inc(sem, 16)
    nc.gpsimd.wait_ge(sem, 32)
    nc.gpsimd.collective_compute(
        "AllGather", mybir.AluOpType.bypass, replica_groups=groups,
        ins=[w1tmp[:, :, :].opt()], outs=[w1g[:, :, :, :].opt()]).then_inc(sem, 1)
    nc.gpsimd.collective_compute(
        "AllGather", mybir.AluOpType.bypass, replica_groups=groups,
        ins=[w2tmp[:, :, :].opt()], outs=[w2g[:, :, :, :].opt()]).then_inc(sem, 1)
    nc.gpsimd.wait_ge(sem, 34)
    nc.all_engine_barrier()
    nc.clear_and_free_semaphores([sem])
    nc.all_engine_barrier()

    # tap groups for batched transpose->copy; PSUM spends a 32-bit word
    # per element even for bf16, so 4x128 per bank
    TGROUPS = ((0, 4), (4, 4), (8, 1))

    with TileContext(nc) as tc:
        with (
            tc.tile_pool(name="wts", bufs=1) as wts,
            tc.tile_pool(name="rows", bufs=4) as rows,
            tc.tile_pool(name="pats", bufs=3) as pats,
            tc.tile_pool(name="mid", bufs=4) as mid,
            tc.tile_pool(name="sml", bufs=3) as sml,
            tc.tile_pool(name="psw1", bufs=1, space="PSUM") as psw1,
            tc.tile_pool(name="psw2", bufs=2, space="PSUM") as psw2,
            tc.tile_pool(name="pst", bufs=2, space="PSUM") as pst,
        ):
            # gathered weights: f1 = rank*F1S + j contiguous; w2 ranks as
            # an explicit axis (rank r holds output-cols [F2S*r, F2S*(r+1)))
            w1k = wts.tile([128, K2, F1], BF_DT)
            w1kv = w1k[:, :, :].rearrange("p k (r j) -> p k r j", r=N_CORES)
            w2k = wts.tile([128, K2, N_CORES, F2S], BF_DT)
            for r in range(N_CORES):
                nc.sync.dma_start(
                    out=w1kv[:, :, r, :],
                    in_=w1g[r, :, :, :].rearrange("k p j -> p k j"))
                nc.sync.dma_start(
                    out=w2k[:, :, r, :],
                    in_=w2g[r, :, :, :].rearrange("k p f -> p k f"))
            b1s = wts.tile([1, F1], BF_DT)
            nc.sync.dma_start(out=b1s, in_=b1_d)
            b2s = wts.tile([1, F2], BF_DT)
            nc.sync.dma_start(out=b2s, in_=b2_d)
            ident = wts.tile([128, 128], BF_DT)
            nc.sync.dma_start(out=ident, in_=id_d)
            ones = wts.tile([1, 128], BF_DT)
            nc.vector.memset(ones, 1.0)

            for h in range(ROWS):
                xmr = rows.tile([128, 3, W + 2], BF_DT)
                nc.sync.dma_start(out=xmr, in_=xm_d[:, h:h + 3, :])
                sr = rows.tile([64, 3, W + 2], BF_DT, tag="srow")
                nc.sync.dma_start(out=sr, in_=s_d[:, h:h + 3, :])

                # on-device patch build: per tap, PE-transpose the padded row
                # slice into [pos, chan] (bf16 PSUM), then batched copy to
                # SBUF; k-major free layout
                xm_pt = pats.tile([128, K2, 128], BF_DT)
                s_pt = pats.tile([128, K2, C], BF_DT)
                for g0, gn in TGROUPS:
                    ptx = pst.tile([128, 4, 128], BF_DT, tag="pt")
                    for j in range(gn):
                        kh, kw = divmod(g0 + j, 3)
                        nc.tensor.transpose(
                            ptx[:, j, :], xmr[:, kh, kw:kw + 128], ident)
                    nc.scalar.copy(out=xm_pt[:, g0:g0 + gn, :],
                                   in_=ptx[:, 0:gn, :])
                for g0, gn in TGROUPS:
                    pts = pst.tile([128, 4, 128], BF_DT, tag="pt")
                    for j in range(gn):
                        kh, kw = divmod(g0 + j, 3)
                        nc.tensor.transpose(
                            pts[:, j, 0:C], sr[:, kh, kw:kw + 128],
                            ident[0:C, 0:C])
                    nc.scalar.copy(out=s_pt[:, g0:g0 + gn, :],
                                   in_=pts[:, 0:gn, 0:C])

                # ---- w1 = feat @ W1 + b1  -> PSUM [128 pos, 576]
                ps1 = psw1.tile([128, F1], F32)
                for lo, hi in ((0, 512), (512, F1)):
                    for k in range(K2):
                        kh, kw = divmod(k, 3)
                        nc.tensor.matmul(
                            ps1[:, lo:hi], xmr[:, kh, kw:kw + 128],
                            w1k[:, k, lo:hi], start=(k == 0), stop=False)
                    nc.tensor.matmul(ps1[:, lo:hi], ones[0:1, :],
                                     b1s[0:1, lo:hi], start=False, stop=True)
                w1b = mid.tile([128, F1], BF_DT)
                nc.scalar.copy(out=w1b, in_=ps1)
                w1v = w1b[:, :].rearrange("p (k c) -> p k c", c=C)

                # ---- yr/mr/sr: per-position reduce over the 9 taps
                rmap = []
                for ci, (pat, absv) in enumerate(
                        ((xm_pt[:, :, 0:C], None),
                         (xm_pt[:, :, C:2 * C], True),
                         (s_pt[:, :, :], True))):
                    t1 = mid.tile([128, K2, C], BF_DT)
                    nc.gpsimd.tensor_mul(t1, pat, w1v)
                    red = sml.tile([128, C], F32, tag=f"red{ci}")
                    nc.vector.tensor_reduce(
                        out=red, in_=t1[:, :, :].rearrange("p k c -> p c k"),
                        axis=mybir.AxisListType.X, op=mybir.AluOpType.add,
                        apply_absolute_value=absv)
                    redb = sml.tile([128, C], BF_DT, tag=f"redb{ci}")
                    nc.scalar.copy(out=redb, in_=red)
                    rmap.append(redb)
                yrb, mrb, srb = rmap

                y_acc = sml.tile([128, C], F32)
                m_acc = sml.tile([128, C], F32)
                s_acc = sml.tile([128, C], F32)

                # ---- w2 = feat @ W2 + b2, 4 chunks of 1024 cols ([o,c] layout)
                for q in range(4):
                    ps2 = psw2.tile([128, 1024], F32)
                    for j2 in range(2):
                        r = q * 2 + j2
                        for k in range(K2):
                            kh, kw = divmod(k, 3)
                            nc.tensor.matmul(
                                ps2[:, j2 * 512:(j2 + 1) * 512],
                                xmr[:, kh, kw:kw + 128],
                                w2k[:, k, r, :], start=(k == 0), stop=False)
                        nc.tensor.matmul(
                            ps2[:, j2 * 512:(j2 + 1) * 512], ones[0:1, :],
                            b2s[0:1, r * F2S:(r + 1) * F2S],
                            start=False, stop=True)
                    w2b = mid.tile([128, 1024], BF_DT)
                    nc.scalar.copy(out=w2b, in_=ps2)
                    w2v = w2b[:, :].rearrange("p (o c) -> p o c", c=C)
                    for redb, acc, absv in ((yrb, y_acc, None),
                                            (mrb, m_acc, True),
                                            (srb, s_acc, True)):
                        t2 = mid.tile([128, 16, C], BF_DT)
                        bcast = redb[:, :].rearrange(
                            "p (o c) -> p o c", o=1).to_broadcast([128, 16, C])
                        # s-chain always on gpsimd; m-chain alternates to
                        # balance DVE vs gpsimd busy time
                        on_q7 = redb is srb or (redb is mrb and q % 2 == 0)
                        eng = nc.gpsimd if on_q7 else nc.vector
                        eng.tensor_mul(t2, w2v, bcast)
                        nc.vector.tensor_reduce(
                            out=acc[:, q * 16:(q + 1) * 16], in_=t2,
                            axis=mybir.AxisListType.X, op=mybir.AluOpType.add,
                            apply_absolute_value=absv)

                srec = sml.tile([128, C], F32)
                nc.vector.reciprocal(out=srec, in_=s_acc)
                my_t = sml.tile([128, C], F16)
                nc.vector.tensor_mul(my_t, m_acc, srec)
                y16 = sml.tile([128, C], F16)
                nc.scalar.copy(out=y16, in_=y_acc)
                nc.sync.dma_start(out=ym_d[h, 0, :, :], in_=y16)
                nc.sync.dma_start(out=ym_d[h, 1, :, :], in_=my_t)
    _split_multi_sync(nc)
    return nc


def _row_gather(Wm, k):
    # rows of W (1152) feeding tap k for channels [x 0..63, m 0..63]
    idx = np.concatenate([np.arange(64) * 9 + k, 576 + np.arange(64) * 9 + k])
    return Wm[idx]


def kernel(x, m, s, W1, b1, W2, b2):
    x = np.asarray(x, np.float32); m = np.asarray(m, np.float32)
    s = np.asarray(s, np.float32)
    W1 = np.asarray(W1, np.float32); W2 = np.asarray(W2, np.float32)
    b1 = np.asarray(b1, np.float32); b2 = np.asarray(b2, np.float32)

    # W1 cols permuted from [c,k] to [k,c] (k-major matches on-device patches)
    W1p = W1.reshape(1152, C, K2).transpose(0, 2, 1).reshape(1152, F1)
    b1p = b1.reshape(C, K2).T.reshape(1, F1).astype(BF)
    # W2 cols permuted from [c,o] to [o,c]; bias likewise
    W2p = W2.reshape(1152, C, C).transpose(0, 2, 1).reshape(1152, F2)
    b2p = b2.reshape(C, C).T.reshape(1, F2).astype(BF)
    w1k = np.stack([_row_gather(W1p.astype(BF), k) for k in range(K2)])
    w2k = np.stack([_row_gather(W2p.astype(BF), k) for k in range(K2)])

    xmp = np.pad(np.concatenate([x, m], axis=1),
                 ((0, 0), (0, 0), (1, 1), (1, 1)), mode='edge').astype(BF)
    sp = np.pad(s, ((0, 0), (0, 0), (1, 1), (1, 1)), mode='edge').astype(BF)
    ident = np.eye(128, dtype=BF)

    in_maps = []
    shards = []
    for core in range(N_CORES):
        b, half = divmod(core, 2)
        h0 = half * (H // 2)
        shards.append((b, h0))
        blob = np.concatenate([
            xmp[b, :, h0:h0 + ROWS + 2, :].ravel(),
            sp[b, :, h0:h0 + ROWS + 2, :].ravel(),
            w1k[:, :, core * F1S:(core + 1) * F1S].ravel(),
            w2k[:, :, core * F2S:(core + 1) * F2S].ravel(),
            b1p.ravel(), b2p.ravel(), ident.ravel(),
        ])
        assert blob.shape[0] == BLOB_N
        in_maps.append({"blob": blob})

    nc = build_program()
    res = run_bass_kernel_spmd(nc, in_maps, core_ids=list(range(N_CORES)),
                               trace=False)
    if os.environ.get("KERNEL_TIME"):
        # no NTFF profiling in this axon build: approximate device time by
        # wall-timing a repeat execution (includes host I/O, so upper bound)
        import time
        from concourse import bass2jax
        t0 = time.time()
        bass2jax.run_bass_via_pjrt(nc, in_maps, n_cores=N_CORES)
        with open("/tmp/kernel_exec_time.txt", "w") as f:
            f.write(str(int((time.time() - t0) * 1e9)))

    y = np.zeros((B, C, H, W), np.float32)
    m_y = np.zeros((B, C, H, W), np.float32)
    for core, (b, h0) in enumerate(shards):
        ym = res.results[core]["ym"]
        y[b, :, h0:h0 + ROWS, :] = ym[:, 0].transpose(2, 0, 1).astype(np.float32)
        m_y[b, :, h0:h0 + ROWS, :] = ym[:, 1].transpose(2, 0, 1).astype(np.float32)
    return y, m_y, np.ones_like(m_y)
